# revision 1
# baseline (speedup 1.0000x reference)
"""Trainium2 Bass kernel for causal multi-head attention (B=4, T=2048, C=1024, H=16).

Sharding (8 cores, zero collectives): core c handles batch b=c//2 and head-half
half=c%2 (8 heads).  Each core:
  1. Q^T/K^T proj for its 8 heads over all T rows (lhsT=W chunk, rhs=xT chunk),
     V proj in natural [k, d] layout (lhsT=xT chunk, rhs=Wv)
  2. causal flash attention in S^T orientation ([k partitions, q free]):
     scores -> causal mask on diagonal 128x128 blocks via a PE
     matmul-accumulate (S += eye.T @ mask, keeps the DVE out of the
     scores->exp chain) -> exp (ScalarE) -> P^T bf16 -> O^T accumulation with
     a ones-column in V giving softmax row-sums in PSUM row 64 -> normalize
     via K=1 broadcast matmul + DVE multiply; odd heads' y^T halves moved to
     partitions 64-127 by an SBUF->SBUF DMA so the output projection
     contracts K=128 per matmul
  3. partial out^T = Wp_half^T @ y^T  (contraction over this core's 512 cols)
Host: pre-transposes/casts x to x^T bf16 per batch, pre-scales Wq by D^-0.5,
slices weights per core; afterwards sums the two partial outputs per batch and
adds bp.  Biases bq/bk/bv (zeros in the spec) are supported via an augmented
ones-row contraction chunk, enabled only when they are nonzero.
"""

import os
import sys

import numpy as np

for _p in ("/opt/trn_rl_repo", "/root/.axon_site/_ro/trn_rl_repo"):
    if os.path.isdir(_p) and _p not in sys.path:
        sys.path.insert(0, _p)

import ml_dtypes  # noqa: E402

import concourse.bass as bass  # noqa: E402
import concourse.bacc as bacc  # noqa: E402
import concourse.mybir as mybir  # noqa: E402
import concourse.tile as tile  # noqa: E402

BF16 = mybir.dt.bfloat16
F32 = mybir.dt.float32
NEG = -1.0e30

C = 1024     # model dim
HALF = 512   # q/k/v columns per core (8 heads x 64)
HC = 8       # heads per core
D = 64       # head dim

_NC_CACHE: dict = {}


def _build_program(kc: int, T: int):
    """Single-core SPMD program.  kc = # of 128-row contraction chunks for the
    QKV projections (8, or 9 when biases are folded via an augmented row)."""
    nc = bacc.Bacc("TRN2", target_bir_lowering=False)

    xT = nc.dram_tensor("xT", [kc * 128, T], BF16, kind="ExternalInput")
    wq = nc.dram_tensor("wq", [kc * 128, HALF], BF16, kind="ExternalInput")
    wk = nc.dram_tensor("wk", [kc * 128, HALF], BF16, kind="ExternalInput")
    wv = nc.dram_tensor("wv", [kc * 128, HALF], BF16, kind="ExternalInput")
    wp = nc.dram_tensor("wp", [HALF, C], BF16, kind="ExternalInput")
    mask = nc.dram_tensor("mask", [128, 128], BF16, kind="ExternalInput")
    eye = nc.dram_tensor("eye", [128, 128], BF16, kind="ExternalInput")
    ones = nc.dram_tensor("ones", [1, 64], BF16, kind="ExternalInput")
    outT = nc.dram_tensor("outT", [C, T], F32, kind="ExternalOutput")

    nqt = T // 512    # number of 512-wide query tiles
    nkr = T // 128    # number of 128-row key chunks

    with tile.TileContext(nc) as tc:
        with (
            tc.tile_pool(name="const", bufs=1) as const,
            tc.tile_pool(name="pt", bufs=16) as ptp,
            tc.tile_pool(name="rnorm", bufs=3) as rnp,
            tc.tile_pool(name="outb", bufs=3) as obp,
            tc.tile_pool(name="ps_s", bufs=2, space="PSUM") as pss,
            tc.tile_pool(name="ps_w", bufs=1, space="PSUM") as psw,
            tc.tile_pool(name="ps_o", bufs=2, space="PSUM") as pso,
        ):
            xt_sb = const.tile([128, kc, T], BF16, tag="xt")
            wq_sb = const.tile([128, kc, HALF], BF16, tag="wq")
            wk_sb = const.tile([128, kc, HALF], BF16, tag="wk")
            wv_sb = const.tile([128, kc, HALF], BF16, tag="wv")
            wp_sb = const.tile([128, 4, C], BF16, tag="wp")
            mask_sb = const.tile([128, 128], BF16, tag="mask")
            eye_sb = const.tile([128, 128], BF16, tag="eye")
            ones_sb = const.tile([65, 64], BF16, tag="ones")
            kt_sb = const.tile([128, 4, T], BF16, tag="kt")
            qt_sb = const.tile([128, 4, T], BF16, tag="qt")
            vx_sb = const.tile([128, nkr, HC, 65], BF16, tag="vx")
            yt_sb = [const.tile([128, T], BF16, tag=f"yt{i}", name=f"yt{i}")
                     for i in range(HC // 2)]

            # input DMAs
            nc.sync.dma_start(
                out=xt_sb[:], in_=xT[:, :].rearrange("(k p) t -> p k t", p=128))
            nc.sync.dma_start(
                out=wq_sb[:], in_=wq[:, :].rearrange("(k p) n -> p k n", p=128))
            nc.sync.dma_start(
                out=wk_sb[:], in_=wk[:, :].rearrange("(k p) n -> p k n", p=128))
            nc.sync.dma_start(
                out=wv_sb[:], in_=wv[:, :].rearrange("(k p) n -> p k n", p=128))
            nc.sync.dma_start(
                out=wp_sb[:], in_=wp[:, :].rearrange("(k p) n -> p k n", p=128))
            nc.sync.dma_start(out=mask_sb[:], in_=mask[:, :])
            nc.sync.dma_start(out=eye_sb[:], in_=eye[:, :])
            nc.sync.dma_start(out=ones_sb[64:65, :], in_=ones[:, :])

            nc.vector.memset(vx_sb[:, :, :, 64:65], 1.0)

            # ---- QKV projections ----
            def proj_t(w_sb, dst_sb, ns):
                # dst[m-chunk 128 (=2 heads), q 512] += W[:,m]^T x^T
                for m in range(4):
                    for n in ns:
                        ps = pss.tile([128, 512], F32, tag="smega")
                        for k in range(kc):
                            nc.tensor.matmul(
                                ps[:, :],
                                w_sb[:, k, 128 * m:128 * m + 128],
                                xt_sb[:, k, 512 * n:512 * n + 512],
                                start=(k == 0), stop=(k == kc - 1))
                        nc.vector.tensor_copy(
                            dst_sb[:, m, 512 * n:512 * n + 512], ps[:, :])

            def proj_t_group(w_sb, dst_sb, m, n):
                ps = psw.tile([128, 512], F32, tag="work")
                for k in range(kc):
                    nc.tensor.matmul(
                        ps[:, :],
                        w_sb[:, k, 128 * m:128 * m + 128],
                        xt_sb[:, k, 512 * n:512 * n + 512],
                        start=(k == 0), stop=(k == kc - 1))
                nc.vector.tensor_copy(
                    dst_sb[:, m, 512 * n:512 * n + 512], ps[:, :])

            proj_t(wq_sb, qt_sb, [0])
            proj_t(wk_sb, kt_sb, [0])

            def proj_v(kr, pool=None, tag=None):
                ps = (pool or psw).tile([128, 512], F32, tag=tag or "work")
                for k in range(kc):
                    nc.tensor.matmul(
                        ps[:, :],
                        xt_sb[:, k, 128 * kr:128 * kr + 128],
                        wv_sb[:, k, :],
                        start=(k == 0), stop=(k == kc - 1))
                nc.vector.tensor_copy(
                    vx_sb[:, kr, :, 0:64], ps[:, :].rearrange("p (h e) -> p h e", e=64))

            for kr in range(min(4, nkr)):
                proj_v(kr, pool=pss, tag="smega")

            def attention(h, qt):
                nch = 4 * qt + 4      # causal key chunks for this q tile
                po = 64 * (h % 2)
                mch = h // 2
                ot = pso.tile([65, 512], F32, tag="o")
                chunk_groups = [list(range(g, min(g + 2, nch)))
                                for g in range(0, nch, 2)]
                for chunks in chunk_groups:
                    ng = len(chunks)
                    sm = pss.tile([128, 512 * ng], F32, tag="smega")
                    geo = []
                    for b, j in enumerate(chunks):
                        dj = j - 4 * qt
                        diag = dj >= 0
                        qo = 128 * dj if diag else 0
                        N = 512 - qo
                        geo.append((b, j, qo, N))
                        nc.tensor.matmul(
                            sm[:, 512 * b:512 * b + N],
                            kt_sb[po:po + 64, mch, 128 * j:128 * j + 128],
                            qt_sb[po:po + 64, mch,
                                  512 * qt + qo:512 * qt + 512],
                            start=True, stop=not diag)
                        if diag:
                            # causal mask on the first 128 cols via a PE
                            # matmul-accumulate: S += eye.T @ mask = mask
                            nc.tensor.matmul(
                                sm[:, 512 * b:512 * b + 128],
                                eye_sb[:, :], mask_sb[:, :],
                                start=False, stop=True)
                    pt = ptp.tile([128, 512 * ng], BF16, tag="pt")
                    nc.scalar.activation(
                        out=pt[:], in_=sm[:, :],
                        func=mybir.ActivationFunctionType.Exp)
                    for b, j, qo, N in geo:
                        nc.tensor.matmul(
                            ot[:, qo:qo + N],
                            vx_sb[:, j, h, :],
                            pt[:, 512 * b:512 * b + N],
                            start=(j == 0), stop=(j == nch - 1))
                # normalize: row 64 of ot holds the softmax denominators
                rc = rnp.tile([65, 512], BF16, tag="recip")
                with nc.allow_low_precision(reason="softmax denom recip in bf16"):
                    nc.vector.reciprocal(rc[64:65, :], ot[64:65, :])
                bc = psw.tile([64, 512], F32, tag="bc")
                nc.tensor.matmul(bc[:, :], ones_sb[64:65, :], rc[64:65, :],
                                 start=True, stop=True)
                rb = rnp.tile([64, 512], F32, tag="rb")
                nc.vector.tensor_copy(rb[:], bc[:, :])
                if h % 2 == 0:
                    nc.vector.tensor_mul(
                        yt_sb[h // 2][0:64, 512 * qt:512 * qt + 512],
                        ot[0:64, :], rb[:])
                else:
                    yto = rnp.tile([64, 512], BF16, tag="yto")
                    nc.vector.tensor_mul(yto[:], ot[0:64, :], rb[:])
                    nc.sync.dma_start(
                        out=yt_sb[h // 2][64:128, 512 * qt:512 * qt + 512],
                        in_=yto[:])

            def proj_out(qt, spread=False):
                for m in range(8):
                    # after the last attention tile the score slots are idle;
                    # rotate the final projection across all three pools
                    if spread and m % 3 != 0:
                        ps = pss.tile([128, 512], F32, tag="smega")
                    else:
                        ps = psw.tile([128, 512], F32, tag="work")
                    for k in range(4):
                        nc.tensor.matmul(
                            ps[:, :],
                            wp_sb[:, k, 128 * m:128 * m + 128],
                            yt_sb[k][:, 512 * qt:512 * qt + 512],
                            start=(k == 0), stop=(k == 3))
                    ob = obp.tile([128, 512], F32, tag="ob")
                    nc.vector.tensor_copy(ob[:], ps[:, :])
                    nc.sync.dma_start(
                        out=outT[128 * m:128 * m + 128,
                                 512 * qt:512 * qt + 512],
                        in_=ob[:])

            # remaining QKV work, emitted interleaved between attention heads
            # so it fills PE idle while ScalarE runs exp
            fillers = []
            for n in range(1, nqt):
                for m in range(4):
                    fillers.append(
                        lambda m=m, n=n: proj_t_group(wk_sb, kt_sb, m, n))
                for m in range(4):
                    fillers.append(
                        lambda m=m, n=n: proj_t_group(wq_sb, qt_sb, m, n))
                for kr in range(4 * n, 4 * n + 4):
                    if kr < nkr:
                        fillers.append(lambda kr=kr: proj_v(kr))

            def emit_fillers(k):
                while k > 0 and fillers:
                    fillers.pop(0)()
                    k -= 1

            for qt in range(nqt):
                if qt > 0:
                    emit_fillers(len(fillers) - 12 * (nqt - 1 - qt))
                for h in range(HC):
                    attention(h, qt)
                    emit_fillers(2 if qt == 0 else 2)
                    if h == 2 and qt > 0:
                        # previous q tile's output projection, emitted here so
                        # its PSUM/PE use hides under this tile's exp chain
                        proj_out(qt - 1)
                if qt == nqt - 1:
                    emit_fillers(len(fillers))
                    proj_out(qt, spread=True)

    nc.finalize()
    return nc


def _prep_inputs(x, Wq, bq, Wk, bk, Wv, bv, Wp, bp, T):
    """Builds per-core in_maps.  Returns (in_maps, kc, use_bias)."""
    bf = ml_dtypes.bfloat16
    scale = 1.0 / np.sqrt(D)
    use_bias = bool(np.any(bq) or np.any(bk) or np.any(bv))
    kc = 9 if use_bias else 8

    mask_np = np.where(np.arange(128)[None, :] >= np.arange(128)[:, None],
                       np.float32(0.0), np.float32(NEG)).astype(bf)
    eye_np = np.eye(128, dtype=np.float32).astype(bf)
    ones_np = np.ones((1, 64), dtype=np.float32).astype(bf)

    def aug_x(xt):  # [1024, T] -> [kc*128, T]
        if not use_bias:
            return xt
        pad = np.zeros((128, xt.shape[1]), dtype=xt.dtype)
        pad[0, :] = 1.0
        return np.concatenate([xt, pad], axis=0)

    def aug_w(w, b):  # [1024, 512] -> [kc*128, 512]
        if not use_bias:
            return w
        pad = np.zeros((128, w.shape[1]), dtype=w.dtype)
        pad[0, :] = b
        return np.concatenate([w, pad], axis=0)

    in_maps = []
    for core in range(8):
        b = core // 2
        half = core % 2
        cs = slice(HALF * half, HALF * half + HALF)
        xt = np.ascontiguousarray(x[b, :T, :].T).astype(np.float32)
        in_maps.append({
            "xT": aug_x(xt).astype(bf),
            "wq": aug_w(Wq[:, cs] * scale, bq[cs] * scale).astype(bf),
            "wk": aug_w(Wk[:, cs], bk[cs]).astype(bf),
            "wv": aug_w(Wv[:, cs], bv[cs]).astype(bf),
            "wp": Wp[cs, :].astype(bf),
            "mask": mask_np,
            "eye": eye_np,
            "ones": ones_np,
        })
    return in_maps, kc, use_bias


def run(inputs: dict, T: int = 2048, trace: bool = False, tmpdir=None):
    """Returns (output [B,T,C] f32, BassKernelResults)."""
    from concourse.bass_utils import run_bass_kernel_spmd

    x = np.asarray(inputs["x"], dtype=np.float32)
    B = x.shape[0]
    in_maps, kc, _ = _prep_inputs(
        x, *[np.asarray(inputs[k], dtype=np.float32) for k in
             ("Wq", "bq", "Wk", "bk", "Wv", "bv", "Wp", "bp")], T)

    key = (kc, T)
    if key not in _NC_CACHE:
        _NC_CACHE[key] = _build_program(kc, T)
    nc = _NC_CACHE[key]

    res = run_bass_kernel_spmd(nc, in_maps, list(range(8)),
                               trace=trace, tmpdir=tmpdir)

    bp = np.asarray(inputs["bp"], dtype=np.float32)
    out = np.empty((B, T, C), dtype=np.float32)
    for b in range(B):
        acc = res.results[2 * b]["outT"] + res.results[2 * b + 1]["outT"]
        out[b] = acc.T + bp[None, :]
    return out, res


def kernel(**inputs) -> np.ndarray:
    out, _ = run(inputs, T=2048, trace=False)
    return out

